# revision 15
# baseline (speedup 1.0000x reference)
"""EnhancedTernaryLinear on 8 Trainium2 NeuronCores.

out = (x @ W^T) * scale + bias
  x: [4, 2048, 4096] f32, W: [4096, 4096] ternary int8, scale/bias: [4096] f32

Strategy: data-parallel over tokens (8192 tokens -> 1024/core), W replicated.
Per core this is a [2048-o x 1024-t x 4096-k] GEMM chunk pipeline:
  - host provides xT [K, T] (k-major) and WT [K, O] so the contraction dim k
    sits on SBUF partitions for both matmul operands
  - x cast f32->bf16 on ScalarE, W cast int8->bf16 on VectorE
  - PE: psum[o=128, t=512] accumulated over 32 k-tiles (bf16 matmul)
  - ScalarE: out = Identity(psum * scale[o] + bias[o]) with per-partition
    scale/bias vectors, f32 out
  - out stored [O, T] per core; host transposes/concats back to [B, S, O]
"""

import numpy as np

B, S, IN_F, OUT_F = 4, 2048, 4096, 4096
N_CORES = 8
TOKENS = B * S
T_PER_CORE = TOKENS // N_CORES

P = 128


def _make_tile_context(nc):
    """TileContext whose end-of-kernel drain splits its sem waits.

    The stock ``_drain_and_barrier`` attaches one wait per logical proc to a
    single SP Drain; the walrus build in this container caps sync waits per
    instruction and rejects that ("Too many sync wait commands").  Emit the
    waits as individual EventSemaphore instructions instead (same semantics:
    SP blocks on each before joining the end-of-kernel barrier).
    """
    import bass_rust
    import concourse.mybir as mybir
    import concourse.tile as tile
    from concourse.vector_clock import ScopedClock

    class SplitDrainTileContext(tile.TileContext):
        def _commit_instruction(self, inst, lazy_reg_writes=True):
            si = inst.sync_info
            if si is not None and si.on_wait:
                cap = 2 if isinstance(inst, mybir.InstEventSemaphore) else 1
                waits = list(si.on_wait)
                if len(waits) > cap:
                    keep, excess = waits[:cap], waits[cap:]
                    for i in range(0, len(excess), 2):
                        chunk = excess[i:i + 2]
                        ev = mybir.InstEventSemaphore(
                            name=self.nc.get_next_instruction_name(),
                            ins=[],
                            outs=[],
                        )
                        ev.engine = inst.engine
                        ev.sync_info = mybir.SyncInfo(
                            on_wait=list(chunk), on_update=[]
                        )
                        super()._commit_instruction(ev)
                    si.on_wait.clear()
                    for w in keep:
                        si.on_wait.append(w)
            return super()._commit_instruction(inst, lazy_reg_writes)

        def _drain_and_barrier(self, tick_clock, wait_clock):
            nc = self.nc
            drain_inst = nc.sync.drain()
            wait_clock.add_sem_waits(
                drain_inst.ins, ScopedClock({None: tick_clock.global_clock})
            )
            si = drain_inst.ins.sync_info
            waits = list(si.on_wait) if si is not None and si.on_wait else []
            if len(waits) > 1:
                si.on_wait.clear()
                for i in range(0, len(waits), 2):
                    ev = mybir.InstEventSemaphore(
                        name=nc.get_next_instruction_name(), ins=[], outs=[]
                    )
                    ev.sync_info = mybir.SyncInfo(
                        on_wait=list(waits[i:i + 2]), on_update=[]
                    )
                    nc.sync.add_instruction(ev)

            nc.all_engine_barrier()
            assert self.sems is not None
            popped = nc._tile_sem_poison_stack.pop()
            assert popped is self._sem_poison
            nc.clear_and_free_semaphores(list(self.sems.allocated().values()))
            nc.all_engine_barrier()

    return SplitDrainTileContext(nc)


def _build(K, O, T, n_wres_bufs=None):
    """Build the single-core Bass program for a [O x T x K] GEMM shard."""
    import concourse.bass as bass
    import concourse.mybir as mybir

    KT = K // P               # k tiles (contraction)
    NT = min(512, T)          # moving free dim per matmul
    TCH = T // NT             # t chunks
    OSUP_W = min(512, O)      # o columns per W staging load
    OSUP = O // OSUP_W
    OSUB = OSUP_W // P        # o tiles per W staging load
    OJ = O // P               # total o tiles

    if n_wres_bufs is None:
        n_wres_bufs = 2 * KT  # double-buffer W super-tiles across osup

    nc = bass.Bass()
    xt_d = nc.declare_dram_parameter("xt", [K, T], mybir.dt.float32, isOutput=False)
    wt_d = nc.declare_dram_parameter("wt", [K, O], mybir.dt.int8, isOutput=False)
    sc_d = nc.declare_dram_parameter("scale2", [P, OJ], mybir.dt.float32, isOutput=False)
    bi_d = nc.declare_dram_parameter("bias2", [P, OJ], mybir.dt.float32, isOutput=False)
    out_d = nc.declare_dram_parameter("out", [O, T], mybir.dt.float32, isOutput=True)

    with _make_tile_context(nc) as tc:
        with (
            tc.tile_pool(name="consts", bufs=1) as consts,
            tc.tile_pool(name="xstage", bufs=4) as xstage,
            tc.tile_pool(name="xres", bufs=KT) as xres,
            tc.tile_pool(name="wstage", bufs=8) as wstage,
            tc.tile_pool(name="wres", bufs=n_wres_bufs) as wres,
            tc.tile_pool(name="outp", bufs=8) as outp,
            tc.tile_pool(name="psum", bufs=8, space="PSUM") as psump,
        ):
            scale_sb = consts.tile([P, OJ], mybir.dt.float32)
            bias_sb = consts.tile([P, OJ], mybir.dt.float32)

            def load_w_tile(osup, kt):
                ws = wstage.tile([P, OSUP_W], mybir.dt.int8)
                nc.sync.dma_start(
                    ws[:],
                    wt_d[kt * P:(kt + 1) * P, osup * OSUP_W:(osup + 1) * OSUP_W],
                )
                wb = wres.tile([P, OSUP_W], mybir.dt.bfloat16)
                nc.vector.tensor_copy(wb[:], ws[:])
                return wb

            def drain_group(ps, j, tch):
                ot = outp.tile([P, NT], mybir.dt.float32)
                nc.scalar.activation(
                    ot[:],
                    ps[:],
                    mybir.ActivationFunctionType.Identity,
                    bias=bias_sb[:, j:j + 1],
                    scale=scale_sb[:, j:j + 1],
                )
                # ACT hwdge queue: keeps the Sync queue free of out-stores,
                # which would otherwise head-of-line-block later W loads
                # behind their ACT-drain data dependency.
                nc.scalar.dma_start(
                    out_d[j * P:(j + 1) * P, tch * NT:(tch + 1) * NT], ot[:]
                )

            # Startup: interleave W(osup=0) and x loads per k-tile so PE can
            # begin immediately; x streams in once and stays resident (bf16).
            xts = []
            wts0 = []
            for kt in range(KT):
                xs = xstage.tile([P, T], mybir.dt.float32)
                xb = xres.tile([P, T], mybir.dt.bfloat16)
                if kt < 2:
                    # split the first tiles so the first matmul's exact
                    # rhs slice lands as early as possible
                    for h in range(TCH):
                        sl = slice(h * NT, (h + 1) * NT)
                        nc.sync.dma_start(xs[:, sl], xt_d[kt * P:(kt + 1) * P, sl])
                        nc.scalar.copy(xb[:, sl], xs[:, sl])
                else:
                    nc.sync.dma_start(xs[:], xt_d[kt * P:(kt + 1) * P, :])
                    nc.scalar.copy(xb[:], xs[:])
                xts.append(xb)
                wts0.append(load_w_tile(0, kt))

            # scale/bias aren't needed until the first psum drain (~60us in);
            # keep them out of the startup descriptor stream
            nc.sync.dma_start(scale_sb[:], sc_d[:])
            nc.sync.dma_start(bias_sb[:], bi_d[:])

            # o_super 0, k-major: 8 matmuls per arriving x k-tile, so PE
            # tracks the x DMA instead of stalling on the full load.
            ps0 = [
                [
                    psump.tile([P, NT], mybir.dt.float32, tag="ps", name=f"ps0_{a}_{b}")
                    for b in range(TCH)
                ]
                for a in range(OSUB)
            ]
            for kt in range(KT):
                for osub in range(OSUB):
                    for tch in range(TCH):
                        nc.tensor.matmul(
                            ps0[osub][tch][:],
                            wts0[kt][:, osub * P:(osub + 1) * P],
                            xts[kt][:, tch * NT:(tch + 1) * NT],
                            start=(kt == 0),
                            stop=(kt == KT - 1),
                        )
            for osub in range(OSUB):
                for tch in range(TCH):
                    drain_group(ps0[osub][tch], osub, tch)

            # o_supers 1..: x is resident; group-major keeps steady state
            # gapless (all deps are W-cast + psum-slot release).
            for osup in range(1, OSUP):
                wts = [load_w_tile(osup, kt) for kt in range(KT)]
                for osub in range(OSUB):
                    j = osup * OSUB + osub
                    for tch in range(TCH):
                        ps = psump.tile([P, NT], mybir.dt.float32)
                        for kt in range(KT):
                            nc.tensor.matmul(
                                ps[:],
                                wts[kt][:, osub * P:(osub + 1) * P],
                                xts[kt][:, tch * NT:(tch + 1) * NT],
                                start=(kt == 0),
                                stop=(kt == KT - 1),
                            )
                        drain_group(ps, j, tch)
    return nc


_NC_CACHE = {}


def _get_nc():
    key = (IN_F, OUT_F, T_PER_CORE)
    if key not in _NC_CACHE:
        _NC_CACHE[key] = _build(IN_F, OUT_F, T_PER_CORE)
    return _NC_CACHE[key]


def _prep_inputs(x, weight_ternary, weight_scale, bias):
    x = np.asarray(x)
    weight_ternary = np.asarray(weight_ternary)
    weight_scale = np.asarray(weight_scale)
    bias = np.asarray(bias)

    x2 = np.ascontiguousarray(
        x.reshape(TOKENS, IN_F).astype(np.float32, copy=False).T
    )  # [K, TOKENS]
    wt = np.ascontiguousarray(weight_ternary.astype(np.int8).T)  # [K, O]
    sc = np.ascontiguousarray(
        weight_scale.astype(np.float32, copy=False).reshape(OUT_F // P, P).T
    )  # [P, OJ]
    bi = np.ascontiguousarray(
        bias.astype(np.float32, copy=False).reshape(OUT_F // P, P).T
    )  # [P, OJ]

    in_maps = []
    for c in range(N_CORES):
        in_maps.append(
            {
                "xt": np.ascontiguousarray(
                    x2[:, c * T_PER_CORE:(c + 1) * T_PER_CORE]
                ),
                "wt": wt,
                "scale2": sc,
                "bias2": bi,
            }
        )
    return in_maps


def _assemble(results):
    # each core returns out [O, T_PER_CORE]; tokens are contiguous per core
    out = np.concatenate(
        [np.ascontiguousarray(r["out"].T) for r in results], axis=0
    )  # [TOKENS, O]
    return out.reshape(B, S, OUT_F)


def _run(x, weight_ternary, weight_scale, bias, trace=False, **spmd_kwargs):
    from concourse.bass_utils import run_bass_kernel_spmd

    nc = _get_nc()
    in_maps = _prep_inputs(x, weight_ternary, weight_scale, bias)
    res = run_bass_kernel_spmd(
        nc, in_maps, core_ids=list(range(N_CORES)), trace=trace, **spmd_kwargs
    )
    return _assemble(res.results), res


def kernel(x, weight_ternary, weight_scale, bias):
    out, _ = _run(x, weight_ternary, weight_scale, bias, trace=False)
    return out


# revision 16
# speedup vs baseline: 1.0055x; 1.0055x over previous
"""EnhancedTernaryLinear on 8 Trainium2 NeuronCores.

out = (x @ W^T) * scale + bias
  x: [4, 2048, 4096] f32, W: [4096, 4096] ternary int8, scale/bias: [4096] f32

Strategy: data-parallel over tokens (8192 tokens -> 1024/core), W replicated.
Per core this is a [2048-o x 1024-t x 4096-k] GEMM chunk pipeline:
  - host provides xT [K, T] (k-major) and WT [K, O] so the contraction dim k
    sits on SBUF partitions for both matmul operands
  - x cast f32->bf16 on ScalarE, W cast int8->bf16 on VectorE
  - PE: psum[o=128, t=512] accumulated over 32 k-tiles (bf16 matmul)
  - ScalarE: out = Identity(psum * scale[o] + bias[o]) with per-partition
    scale/bias vectors, f32 out
  - out stored [O, T] per core; host transposes/concats back to [B, S, O]
"""

import numpy as np

B, S, IN_F, OUT_F = 4, 2048, 4096, 4096
N_CORES = 8
TOKENS = B * S
T_PER_CORE = TOKENS // N_CORES

P = 128


def _make_tile_context(nc):
    """TileContext whose end-of-kernel drain splits its sem waits.

    The stock ``_drain_and_barrier`` attaches one wait per logical proc to a
    single SP Drain; the walrus build in this container caps sync waits per
    instruction and rejects that ("Too many sync wait commands").  Emit the
    waits as individual EventSemaphore instructions instead (same semantics:
    SP blocks on each before joining the end-of-kernel barrier).
    """
    import bass_rust
    import concourse.mybir as mybir
    import concourse.tile as tile
    from concourse.vector_clock import ScopedClock

    class SplitDrainTileContext(tile.TileContext):
        def _commit_instruction(self, inst, lazy_reg_writes=True):
            si = inst.sync_info
            if si is not None and si.on_wait:
                cap = 2 if isinstance(inst, mybir.InstEventSemaphore) else 1
                waits = list(si.on_wait)
                if len(waits) > cap:
                    keep, excess = waits[:cap], waits[cap:]
                    for i in range(0, len(excess), 2):
                        chunk = excess[i:i + 2]
                        ev = mybir.InstEventSemaphore(
                            name=self.nc.get_next_instruction_name(),
                            ins=[],
                            outs=[],
                        )
                        ev.engine = inst.engine
                        ev.sync_info = mybir.SyncInfo(
                            on_wait=list(chunk), on_update=[]
                        )
                        super()._commit_instruction(ev)
                    si.on_wait.clear()
                    for w in keep:
                        si.on_wait.append(w)
            return super()._commit_instruction(inst, lazy_reg_writes)

        def _drain_and_barrier(self, tick_clock, wait_clock):
            nc = self.nc
            drain_inst = nc.sync.drain()
            wait_clock.add_sem_waits(
                drain_inst.ins, ScopedClock({None: tick_clock.global_clock})
            )
            si = drain_inst.ins.sync_info
            waits = list(si.on_wait) if si is not None and si.on_wait else []
            if len(waits) > 1:
                si.on_wait.clear()
                for i in range(0, len(waits), 2):
                    ev = mybir.InstEventSemaphore(
                        name=nc.get_next_instruction_name(), ins=[], outs=[]
                    )
                    ev.sync_info = mybir.SyncInfo(
                        on_wait=list(waits[i:i + 2]), on_update=[]
                    )
                    nc.sync.add_instruction(ev)

            nc.all_engine_barrier()
            assert self.sems is not None
            popped = nc._tile_sem_poison_stack.pop()
            assert popped is self._sem_poison
            nc.clear_and_free_semaphores(list(self.sems.allocated().values()))
            nc.all_engine_barrier()

    return SplitDrainTileContext(nc)


def _build(K, O, T, n_wres_bufs=None):
    """Build the single-core Bass program for a [O x T x K] GEMM shard."""
    import concourse.bass as bass
    import concourse.mybir as mybir

    KT = K // P               # k tiles (contraction)
    NT = min(512, T)          # moving free dim per matmul
    TCH = T // NT             # t chunks
    OSUP_W = min(512, O)      # o columns per W staging load
    OSUP = O // OSUP_W
    OSUB = OSUP_W // P        # o tiles per W staging load
    OJ = O // P               # total o tiles

    if n_wres_bufs is None:
        n_wres_bufs = 2 * KT  # double-buffer W super-tiles across osup

    nc = bass.Bass()
    xt_d = nc.declare_dram_parameter("xt", [K, T], mybir.dt.float32, isOutput=False)
    wt_d = nc.declare_dram_parameter("wt", [K, O], mybir.dt.int8, isOutput=False)
    sc_d = nc.declare_dram_parameter("scale2", [P, OJ], mybir.dt.float32, isOutput=False)
    bi_d = nc.declare_dram_parameter("bias2", [P, OJ], mybir.dt.float32, isOutput=False)
    out_d = nc.declare_dram_parameter("out", [O, T], mybir.dt.float32, isOutput=True)

    with _make_tile_context(nc) as tc:
        with (
            tc.tile_pool(name="consts", bufs=1) as consts,
            tc.tile_pool(name="xstage", bufs=4) as xstage,
            tc.tile_pool(name="xres", bufs=KT) as xres,
            tc.tile_pool(name="wstage", bufs=8) as wstage,
            tc.tile_pool(name="wres", bufs=n_wres_bufs) as wres,
            tc.tile_pool(name="outp", bufs=8) as outp,
            tc.tile_pool(name="psum", bufs=8, space="PSUM") as psump,
        ):
            scale_sb = consts.tile([P, OJ], mybir.dt.float32)
            bias_sb = consts.tile([P, OJ], mybir.dt.float32)

            def load_w_tile(osup, kt):
                ws = wstage.tile([P, OSUP_W], mybir.dt.int8)
                nc.sync.dma_start(
                    ws[:],
                    wt_d[kt * P:(kt + 1) * P, osup * OSUP_W:(osup + 1) * OSUP_W],
                )
                wb = wres.tile([P, OSUP_W], mybir.dt.bfloat16)
                nc.vector.tensor_copy(wb[:], ws[:])
                return wb

            def drain_group(ps, j, tch):
                ot = outp.tile([P, NT], mybir.dt.float32)
                nc.scalar.activation(
                    ot[:],
                    ps[:],
                    mybir.ActivationFunctionType.Identity,
                    bias=bias_sb[:, j:j + 1],
                    scale=scale_sb[:, j:j + 1],
                )
                # ACT hwdge queue: keeps the Sync queue free of out-stores,
                # which would otherwise head-of-line-block later W loads
                # behind their ACT-drain data dependency.
                nc.scalar.dma_start(
                    out_d[j * P:(j + 1) * P, tch * NT:(tch + 1) * NT], ot[:]
                )

            # Startup: interleave W(osup=0) and x loads per k-tile so PE can
            # begin immediately; x streams in once and stays resident (bf16).
            xts = []
            wts0 = []
            for kt in range(KT):
                wts0.append(load_w_tile(0, kt))
                xs = xstage.tile([P, T], mybir.dt.float32)
                nc.sync.dma_start(xs[:], xt_d[kt * P:(kt + 1) * P, :])
                xb = xres.tile([P, T], mybir.dt.bfloat16)
                nc.scalar.copy(xb[:], xs[:])
                xts.append(xb)

            # scale/bias aren't needed until the first psum drain (~60us in);
            # keep them out of the startup descriptor stream
            nc.sync.dma_start(scale_sb[:], sc_d[:])
            nc.sync.dma_start(bias_sb[:], bi_d[:])

            # o_super 0, k-major: 8 matmuls per arriving x k-tile, so PE
            # tracks the x DMA instead of stalling on the full load.
            ps0 = [
                [
                    psump.tile([P, NT], mybir.dt.float32, tag="ps", name=f"ps0_{a}_{b}")
                    for b in range(TCH)
                ]
                for a in range(OSUB)
            ]
            for kt in range(KT):
                for osub in range(OSUB):
                    for tch in range(TCH):
                        nc.tensor.matmul(
                            ps0[osub][tch][:],
                            wts0[kt][:, osub * P:(osub + 1) * P],
                            xts[kt][:, tch * NT:(tch + 1) * NT],
                            start=(kt == 0),
                            stop=(kt == KT - 1),
                        )
            for osub in range(OSUB):
                for tch in range(TCH):
                    drain_group(ps0[osub][tch], osub, tch)

            # o_supers 1..: x is resident; group-major keeps steady state
            # gapless (all deps are W-cast + psum-slot release).
            for osup in range(1, OSUP):
                wts = [load_w_tile(osup, kt) for kt in range(KT)]
                for osub in range(OSUB):
                    j = osup * OSUB + osub
                    for tch in range(TCH):
                        ps = psump.tile([P, NT], mybir.dt.float32)
                        for kt in range(KT):
                            nc.tensor.matmul(
                                ps[:],
                                wts[kt][:, osub * P:(osub + 1) * P],
                                xts[kt][:, tch * NT:(tch + 1) * NT],
                                start=(kt == 0),
                                stop=(kt == KT - 1),
                            )
                        drain_group(ps, j, tch)
    return nc


_NC_CACHE = {}


def _get_nc():
    key = (IN_F, OUT_F, T_PER_CORE)
    if key not in _NC_CACHE:
        _NC_CACHE[key] = _build(IN_F, OUT_F, T_PER_CORE)
    return _NC_CACHE[key]


def _prep_inputs(x, weight_ternary, weight_scale, bias):
    x = np.asarray(x)
    weight_ternary = np.asarray(weight_ternary)
    weight_scale = np.asarray(weight_scale)
    bias = np.asarray(bias)

    x2 = np.ascontiguousarray(
        x.reshape(TOKENS, IN_F).astype(np.float32, copy=False).T
    )  # [K, TOKENS]
    wt = np.ascontiguousarray(weight_ternary.astype(np.int8).T)  # [K, O]
    sc = np.ascontiguousarray(
        weight_scale.astype(np.float32, copy=False).reshape(OUT_F // P, P).T
    )  # [P, OJ]
    bi = np.ascontiguousarray(
        bias.astype(np.float32, copy=False).reshape(OUT_F // P, P).T
    )  # [P, OJ]

    in_maps = []
    for c in range(N_CORES):
        in_maps.append(
            {
                "xt": np.ascontiguousarray(
                    x2[:, c * T_PER_CORE:(c + 1) * T_PER_CORE]
                ),
                "wt": wt,
                "scale2": sc,
                "bias2": bi,
            }
        )
    return in_maps


def _assemble(results):
    # each core returns out [O, T_PER_CORE]; tokens are contiguous per core
    out = np.concatenate(
        [np.ascontiguousarray(r["out"].T) for r in results], axis=0
    )  # [TOKENS, O]
    return out.reshape(B, S, OUT_F)


def _run(x, weight_ternary, weight_scale, bias, trace=False, **spmd_kwargs):
    from concourse.bass_utils import run_bass_kernel_spmd

    nc = _get_nc()
    in_maps = _prep_inputs(x, weight_ternary, weight_scale, bias)
    res = run_bass_kernel_spmd(
        nc, in_maps, core_ids=list(range(N_CORES)), trace=trace, **spmd_kwargs
    )
    return _assemble(res.results), res


def kernel(x, weight_ternary, weight_scale, bias):
    out, _ = _run(x, weight_ternary, weight_scale, bias, trace=False)
    return out


# revision 18
# speedup vs baseline: 1.0189x; 1.0133x over previous
"""EnhancedTernaryLinear on 8 Trainium2 NeuronCores.

out = (x @ W^T) * scale + bias
  x: [4, 2048, 4096] f32, W: [4096, 4096] ternary int8, scale/bias: [4096] f32

Strategy: data-parallel over tokens (8192 tokens -> 1024/core), W replicated.
Per core this is a [2048-o x 1024-t x 4096-k] GEMM chunk pipeline:
  - host provides xT [K, T] (k-major) and WT [K, O] so the contraction dim k
    sits on SBUF partitions for both matmul operands
  - x cast f32->bf16 on ScalarE, W cast int8->bf16 on VectorE
  - PE: psum[o=128, t=512] accumulated over 32 k-tiles (bf16 matmul)
  - ScalarE: out = Identity(psum * scale[o] + bias[o]) with per-partition
    scale/bias vectors, f32 out
  - out stored [O, T] per core; host transposes/concats back to [B, S, O]
"""

import numpy as np

B, S, IN_F, OUT_F = 4, 2048, 4096, 4096
N_CORES = 8
TOKENS = B * S
T_PER_CORE = TOKENS // N_CORES

P = 128


def _make_tile_context(nc):
    """TileContext whose end-of-kernel drain splits its sem waits.

    The stock ``_drain_and_barrier`` attaches one wait per logical proc to a
    single SP Drain; the walrus build in this container caps sync waits per
    instruction and rejects that ("Too many sync wait commands").  Emit the
    waits as individual EventSemaphore instructions instead (same semantics:
    SP blocks on each before joining the end-of-kernel barrier).
    """
    import bass_rust
    import concourse.mybir as mybir
    import concourse.tile as tile
    from concourse.vector_clock import ScopedClock

    class SplitDrainTileContext(tile.TileContext):
        def _commit_instruction(self, inst, lazy_reg_writes=True):
            si = inst.sync_info
            if si is not None and si.on_wait:
                cap = 2 if isinstance(inst, mybir.InstEventSemaphore) else 1
                waits = list(si.on_wait)
                if len(waits) > cap:
                    keep, excess = waits[:cap], waits[cap:]
                    for i in range(0, len(excess), 2):
                        chunk = excess[i:i + 2]
                        ev = mybir.InstEventSemaphore(
                            name=self.nc.get_next_instruction_name(),
                            ins=[],
                            outs=[],
                        )
                        ev.engine = inst.engine
                        ev.sync_info = mybir.SyncInfo(
                            on_wait=list(chunk), on_update=[]
                        )
                        super()._commit_instruction(ev)
                    si.on_wait.clear()
                    for w in keep:
                        si.on_wait.append(w)
            return super()._commit_instruction(inst, lazy_reg_writes)

        def _drain_and_barrier(self, tick_clock, wait_clock):
            nc = self.nc
            drain_inst = nc.sync.drain()
            wait_clock.add_sem_waits(
                drain_inst.ins, ScopedClock({None: tick_clock.global_clock})
            )
            si = drain_inst.ins.sync_info
            waits = list(si.on_wait) if si is not None and si.on_wait else []
            if len(waits) > 1:
                si.on_wait.clear()
                for i in range(0, len(waits), 2):
                    ev = mybir.InstEventSemaphore(
                        name=nc.get_next_instruction_name(), ins=[], outs=[]
                    )
                    ev.sync_info = mybir.SyncInfo(
                        on_wait=list(waits[i:i + 2]), on_update=[]
                    )
                    nc.sync.add_instruction(ev)

            nc.all_engine_barrier()
            assert self.sems is not None
            popped = nc._tile_sem_poison_stack.pop()
            assert popped is self._sem_poison
            nc.clear_and_free_semaphores(list(self.sems.allocated().values()))
            nc.all_engine_barrier()

    return SplitDrainTileContext(nc)


def _build(K, O, T, n_wres_bufs=None):
    """Build the single-core Bass program for a [O x T x K] GEMM shard."""
    import concourse.bass as bass
    import concourse.mybir as mybir

    KT = K // P               # k tiles (contraction)
    NT = min(512, T)          # moving free dim per matmul
    TCH = T // NT             # t chunks
    OSUP_W = min(512, O)      # o columns per W staging load
    OSUP = O // OSUP_W
    OSUB = OSUP_W // P        # o tiles per W staging load
    OJ = O // P               # total o tiles

    if n_wres_bufs is None:
        n_wres_bufs = 2 * (KT // 4)  # double-buffer W super-tiles across osup

    nc = bass.Bass()
    xt_d = nc.declare_dram_parameter("xt", [K, T], mybir.dt.float32, isOutput=False)
    wt_d = nc.declare_dram_parameter("wt", [K, O], mybir.dt.int8, isOutput=False)
    sc_d = nc.declare_dram_parameter("scale2", [P, OJ], mybir.dt.float32, isOutput=False)
    bi_d = nc.declare_dram_parameter("bias2", [P, OJ], mybir.dt.float32, isOutput=False)
    out_d = nc.declare_dram_parameter("out", [O, T], mybir.dt.float32, isOutput=True)

    with _make_tile_context(nc) as tc:
        with (
            tc.tile_pool(name="consts", bufs=1) as consts,
            tc.tile_pool(name="xstage", bufs=4) as xstage,
            tc.tile_pool(name="xres", bufs=KT) as xres,
            tc.tile_pool(name="wstage", bufs=4) as wstage,
            tc.tile_pool(name="wres", bufs=n_wres_bufs) as wres,
            tc.tile_pool(name="outp", bufs=8) as outp,
            tc.tile_pool(name="psum", bufs=8, space="PSUM") as psump,
        ):
            scale_sb = consts.tile([P, OJ], mybir.dt.float32)
            bias_sb = consts.tile([P, OJ], mybir.dt.float32)

            KB = 4  # k-tiles per batched W load (one DMA descriptor)

            def load_w_batch(osup, kg):
                """Load k-tiles [kg*KB, (kg+1)*KB) of W column block osup as
                one [P, KB, OSUP_W] DMA + cast; returns the bf16 tile."""
                ws = wstage.tile([P, KB, OSUP_W], mybir.dt.int8)
                src = wt_d[
                    kg * KB * P:(kg + 1) * KB * P,
                    osup * OSUP_W:(osup + 1) * OSUP_W,
                ].rearrange("(a p) o -> p a o", p=P)
                nc.sync.dma_start(ws[:], src)
                wb = wres.tile([P, KB, OSUP_W], mybir.dt.bfloat16)
                nc.vector.tensor_copy(wb[:], ws[:])
                return wb

            def w_slice(wbatches, kt, osub):
                return wbatches[kt // KB][:, kt % KB, osub * P:(osub + 1) * P]

            def drain_group(ps, j, tch):
                ot = outp.tile([P, NT], mybir.dt.float32)
                nc.scalar.activation(
                    ot[:],
                    ps[:],
                    mybir.ActivationFunctionType.Identity,
                    bias=bias_sb[:, j:j + 1],
                    scale=scale_sb[:, j:j + 1],
                )
                # ACT hwdge queue: keeps the Sync queue free of out-stores,
                # which would otherwise head-of-line-block later W loads
                # behind their ACT-drain data dependency.
                nc.scalar.dma_start(
                    out_d[j * P:(j + 1) * P, tch * NT:(tch + 1) * NT], ot[:]
                )

            # Startup: interleave W(osup=0) and x loads per k-tile so PE can
            # begin immediately; x streams in once and stays resident (bf16).
            xts = []
            wts0 = []
            for kt in range(KT):
                if kt % KB == 0:
                    wts0.append(load_w_batch(0, kt // KB))
                xs = xstage.tile([P, T], mybir.dt.float32)
                nc.sync.dma_start(xs[:], xt_d[kt * P:(kt + 1) * P, :])
                xb = xres.tile([P, T], mybir.dt.bfloat16)
                nc.scalar.copy(xb[:], xs[:])
                xts.append(xb)

            # scale/bias aren't needed until the first psum drain (~60us in);
            # keep them out of the startup descriptor stream
            nc.sync.dma_start(scale_sb[:], sc_d[:])
            nc.sync.dma_start(bias_sb[:], bi_d[:])

            # o_super 0, k-major: 8 matmuls per arriving x k-tile, so PE
            # tracks the x DMA instead of stalling on the full load.
            ps0 = [
                [
                    psump.tile([P, NT], mybir.dt.float32, tag="ps", name=f"ps0_{a}_{b}")
                    for b in range(TCH)
                ]
                for a in range(OSUB)
            ]
            for kt in range(KT):
                for osub in range(OSUB):
                    for tch in range(TCH):
                        nc.tensor.matmul(
                            ps0[osub][tch][:],
                            w_slice(wts0, kt, osub),
                            xts[kt][:, tch * NT:(tch + 1) * NT],
                            start=(kt == 0),
                            stop=(kt == KT - 1),
                        )
            for osub in range(OSUB):
                for tch in range(TCH):
                    drain_group(ps0[osub][tch], osub, tch)

            # o_supers 1..: x is resident; group-major keeps steady state
            # gapless (all deps are W-cast + psum-slot release).
            for osup in range(1, OSUP):
                wts = [load_w_batch(osup, kg) for kg in range(KT // KB)]
                for osub in range(OSUB):
                    j = osup * OSUB + osub
                    for tch in range(TCH):
                        ps = psump.tile([P, NT], mybir.dt.float32)
                        for kt in range(KT):
                            nc.tensor.matmul(
                                ps[:],
                                w_slice(wts, kt, osub),
                                xts[kt][:, tch * NT:(tch + 1) * NT],
                                start=(kt == 0),
                                stop=(kt == KT - 1),
                            )
                        drain_group(ps, j, tch)
    return nc


_NC_CACHE = {}


def _get_nc():
    key = (IN_F, OUT_F, T_PER_CORE)
    if key not in _NC_CACHE:
        _NC_CACHE[key] = _build(IN_F, OUT_F, T_PER_CORE)
    return _NC_CACHE[key]


def _prep_inputs(x, weight_ternary, weight_scale, bias):
    x = np.asarray(x)
    weight_ternary = np.asarray(weight_ternary)
    weight_scale = np.asarray(weight_scale)
    bias = np.asarray(bias)

    x2 = np.ascontiguousarray(
        x.reshape(TOKENS, IN_F).astype(np.float32, copy=False).T
    )  # [K, TOKENS]
    wt = np.ascontiguousarray(weight_ternary.astype(np.int8).T)  # [K, O]
    sc = np.ascontiguousarray(
        weight_scale.astype(np.float32, copy=False).reshape(OUT_F // P, P).T
    )  # [P, OJ]
    bi = np.ascontiguousarray(
        bias.astype(np.float32, copy=False).reshape(OUT_F // P, P).T
    )  # [P, OJ]

    in_maps = []
    for c in range(N_CORES):
        in_maps.append(
            {
                "xt": np.ascontiguousarray(
                    x2[:, c * T_PER_CORE:(c + 1) * T_PER_CORE]
                ),
                "wt": wt,
                "scale2": sc,
                "bias2": bi,
            }
        )
    return in_maps


def _assemble(results):
    # each core returns out [O, T_PER_CORE]; tokens are contiguous per core
    out = np.concatenate(
        [np.ascontiguousarray(r["out"].T) for r in results], axis=0
    )  # [TOKENS, O]
    return out.reshape(B, S, OUT_F)


def _run(x, weight_ternary, weight_scale, bias, trace=False, **spmd_kwargs):
    from concourse.bass_utils import run_bass_kernel_spmd

    nc = _get_nc()
    in_maps = _prep_inputs(x, weight_ternary, weight_scale, bias)
    res = run_bass_kernel_spmd(
        nc, in_maps, core_ids=list(range(N_CORES)), trace=trace, **spmd_kwargs
    )
    return _assemble(res.results), res


def kernel(x, weight_ternary, weight_scale, bias):
    out, _ = _run(x, weight_ternary, weight_scale, bias, trace=False)
    return out


# revision 19
# speedup vs baseline: 1.0205x; 1.0016x over previous
"""EnhancedTernaryLinear on 8 Trainium2 NeuronCores.

out = (x @ W^T) * scale + bias
  x: [4, 2048, 4096] f32, W: [4096, 4096] ternary int8, scale/bias: [4096] f32

Strategy: data-parallel over tokens (8192 tokens -> 1024/core), W replicated.
Per core this is a [2048-o x 1024-t x 4096-k] GEMM chunk pipeline:
  - host provides xT [K, T] (k-major) and WT [K, O] so the contraction dim k
    sits on SBUF partitions for both matmul operands
  - x cast f32->bf16 on ScalarE, W cast int8->bf16 on VectorE
  - PE: psum[o=128, t=512] accumulated over 32 k-tiles (bf16 matmul)
  - ScalarE: out = Identity(psum * scale[o] + bias[o]) with per-partition
    scale/bias vectors, f32 out
  - out stored [O, T] per core; host transposes/concats back to [B, S, O]
"""

import numpy as np

B, S, IN_F, OUT_F = 4, 2048, 4096, 4096
N_CORES = 8
TOKENS = B * S
T_PER_CORE = TOKENS // N_CORES

P = 128


def _make_tile_context(nc):
    """TileContext whose end-of-kernel drain splits its sem waits.

    The stock ``_drain_and_barrier`` attaches one wait per logical proc to a
    single SP Drain; the walrus build in this container caps sync waits per
    instruction and rejects that ("Too many sync wait commands").  Emit the
    waits as individual EventSemaphore instructions instead (same semantics:
    SP blocks on each before joining the end-of-kernel barrier).
    """
    import bass_rust
    import concourse.mybir as mybir
    import concourse.tile as tile
    from concourse.vector_clock import ScopedClock

    class SplitDrainTileContext(tile.TileContext):
        def _commit_instruction(self, inst, lazy_reg_writes=True):
            si = inst.sync_info
            if si is not None and si.on_wait:
                cap = 2 if isinstance(inst, mybir.InstEventSemaphore) else 1
                waits = list(si.on_wait)
                if len(waits) > cap:
                    keep, excess = waits[:cap], waits[cap:]
                    for i in range(0, len(excess), 2):
                        chunk = excess[i:i + 2]
                        ev = mybir.InstEventSemaphore(
                            name=self.nc.get_next_instruction_name(),
                            ins=[],
                            outs=[],
                        )
                        ev.engine = inst.engine
                        ev.sync_info = mybir.SyncInfo(
                            on_wait=list(chunk), on_update=[]
                        )
                        super()._commit_instruction(ev)
                    si.on_wait.clear()
                    for w in keep:
                        si.on_wait.append(w)
            return super()._commit_instruction(inst, lazy_reg_writes)

        def _drain_and_barrier(self, tick_clock, wait_clock):
            nc = self.nc
            drain_inst = nc.sync.drain()
            wait_clock.add_sem_waits(
                drain_inst.ins, ScopedClock({None: tick_clock.global_clock})
            )
            si = drain_inst.ins.sync_info
            waits = list(si.on_wait) if si is not None and si.on_wait else []
            if len(waits) > 1:
                si.on_wait.clear()
                for i in range(0, len(waits), 2):
                    ev = mybir.InstEventSemaphore(
                        name=nc.get_next_instruction_name(), ins=[], outs=[]
                    )
                    ev.sync_info = mybir.SyncInfo(
                        on_wait=list(waits[i:i + 2]), on_update=[]
                    )
                    nc.sync.add_instruction(ev)

            nc.all_engine_barrier()
            assert self.sems is not None
            popped = nc._tile_sem_poison_stack.pop()
            assert popped is self._sem_poison
            nc.clear_and_free_semaphores(list(self.sems.allocated().values()))
            nc.all_engine_barrier()

    return SplitDrainTileContext(nc)


def _build(K, O, T, n_wres_bufs=None):
    """Build the single-core Bass program for a [O x T x K] GEMM shard."""
    import concourse.bass as bass
    import concourse.mybir as mybir

    KT = K // P               # k tiles (contraction)
    NT = min(512, T)          # moving free dim per matmul
    TCH = T // NT             # t chunks
    OSUP_W = min(512, O)      # o columns per W staging load
    OSUP = O // OSUP_W
    OSUB = OSUP_W // P        # o tiles per W staging load
    OJ = O // P               # total o tiles

    KB = min(4, KT)           # k-tiles per batched W load (one DMA descriptor)
    if n_wres_bufs is None:
        n_wres_bufs = 2 * (KT // KB)  # double-buffer W super-tiles across osup

    nc = bass.Bass()
    xt_d = nc.declare_dram_parameter("xt", [K, T], mybir.dt.float32, isOutput=False)
    wt_d = nc.declare_dram_parameter("wt", [K, O], mybir.dt.int8, isOutput=False)
    sc_d = nc.declare_dram_parameter("scale2", [P, OJ], mybir.dt.float32, isOutput=False)
    bi_d = nc.declare_dram_parameter("bias2", [P, OJ], mybir.dt.float32, isOutput=False)
    out_d = nc.declare_dram_parameter("out", [O, T], mybir.dt.float32, isOutput=True)

    with _make_tile_context(nc) as tc:
        with (
            tc.tile_pool(name="consts", bufs=1) as consts,
            tc.tile_pool(name="xstage", bufs=4) as xstage,
            tc.tile_pool(name="xres", bufs=KT) as xres,
            tc.tile_pool(name="wstage", bufs=4) as wstage,
            tc.tile_pool(name="wres", bufs=n_wres_bufs) as wres,
            tc.tile_pool(name="outp", bufs=8) as outp,
            tc.tile_pool(name="psum", bufs=8, space="PSUM") as psump,
        ):
            scale_sb = consts.tile([P, OJ], mybir.dt.float32)
            bias_sb = consts.tile([P, OJ], mybir.dt.float32)

            def load_w_batch(osup, kg):
                """Load k-tiles [kg*KB, (kg+1)*KB) of W column block osup as
                one [P, KB, OSUP_W] DMA + cast; returns the bf16 tile."""
                ws = wstage.tile([P, KB, OSUP_W], mybir.dt.int8)
                src = wt_d[
                    kg * KB * P:(kg + 1) * KB * P,
                    osup * OSUP_W:(osup + 1) * OSUP_W,
                ].rearrange("(a p) o -> p a o", p=P)
                nc.sync.dma_start(ws[:], src)
                wb = wres.tile([P, KB, OSUP_W], mybir.dt.bfloat16)
                nc.vector.tensor_copy(wb[:], ws[:])
                return wb

            def w_slice(wbatches, kt, osub):
                return wbatches[kt // KB][:, kt % KB, osub * P:(osub + 1) * P]

            def drain_group(ps, j, tch):
                ot = outp.tile([P, NT], mybir.dt.float32)
                nc.scalar.activation(
                    ot[:],
                    ps[:],
                    mybir.ActivationFunctionType.Identity,
                    bias=bias_sb[:, j:j + 1],
                    scale=scale_sb[:, j:j + 1],
                )
                # ACT hwdge queue: keeps the Sync queue free of out-stores,
                # which would otherwise head-of-line-block later W loads
                # behind their ACT-drain data dependency.
                nc.scalar.dma_start(
                    out_d[j * P:(j + 1) * P, tch * NT:(tch + 1) * NT], ot[:]
                )

            # Startup: interleave W(osup=0) and x loads per k-tile so PE can
            # begin immediately; x streams in once and stays resident (bf16).
            xts = []
            wts0 = []
            for kt in range(KT):
                if kt % KB == 0:
                    wts0.append(load_w_batch(0, kt // KB))
                xs = xstage.tile([P, T], mybir.dt.float32)
                nc.sync.dma_start(xs[:], xt_d[kt * P:(kt + 1) * P, :])
                xb = xres.tile([P, T], mybir.dt.bfloat16)
                nc.scalar.copy(xb[:], xs[:])
                xts.append(xb)

            # scale/bias aren't needed until the first psum drain (~60us in);
            # keep them out of the startup descriptor stream
            nc.sync.dma_start(scale_sb[:], sc_d[:])
            nc.sync.dma_start(bias_sb[:], bi_d[:])

            # o_super 0, k-major: 8 matmuls per arriving x k-tile, so PE
            # tracks the x DMA instead of stalling on the full load.
            ps0 = [
                [
                    psump.tile([P, NT], mybir.dt.float32, tag="ps", name=f"ps0_{a}_{b}")
                    for b in range(TCH)
                ]
                for a in range(OSUB)
            ]
            for kt in range(KT):
                for osub in range(OSUB):
                    for tch in range(TCH):
                        nc.tensor.matmul(
                            ps0[osub][tch][:],
                            w_slice(wts0, kt, osub),
                            xts[kt][:, tch * NT:(tch + 1) * NT],
                            start=(kt == 0),
                            stop=(kt == KT - 1),
                        )
            for osub in range(OSUB):
                for tch in range(TCH):
                    drain_group(ps0[osub][tch], osub, tch)

            # o_supers 1..: x is resident; group-major keeps steady state
            # gapless (all deps are W-cast + psum-slot release).
            for osup in range(1, OSUP):
                wts = [load_w_batch(osup, kg) for kg in range(KT // KB)]
                for osub in range(OSUB):
                    j = osup * OSUB + osub
                    for tch in range(TCH):
                        ps = psump.tile([P, NT], mybir.dt.float32)
                        for kt in range(KT):
                            nc.tensor.matmul(
                                ps[:],
                                w_slice(wts, kt, osub),
                                xts[kt][:, tch * NT:(tch + 1) * NT],
                                start=(kt == 0),
                                stop=(kt == KT - 1),
                            )
                        drain_group(ps, j, tch)
    return nc


_NC_CACHE = {}


def _get_nc():
    key = (IN_F, OUT_F, T_PER_CORE)
    if key not in _NC_CACHE:
        _NC_CACHE[key] = _build(IN_F, OUT_F, T_PER_CORE)
    return _NC_CACHE[key]


def _prep_inputs(x, weight_ternary, weight_scale, bias):
    x = np.asarray(x)
    weight_ternary = np.asarray(weight_ternary)
    weight_scale = np.asarray(weight_scale)
    bias = np.asarray(bias)

    x2 = np.ascontiguousarray(
        x.reshape(TOKENS, IN_F).astype(np.float32, copy=False).T
    )  # [K, TOKENS]
    wt = np.ascontiguousarray(weight_ternary.astype(np.int8).T)  # [K, O]
    sc = np.ascontiguousarray(
        weight_scale.astype(np.float32, copy=False).reshape(OUT_F // P, P).T
    )  # [P, OJ]
    bi = np.ascontiguousarray(
        bias.astype(np.float32, copy=False).reshape(OUT_F // P, P).T
    )  # [P, OJ]

    in_maps = []
    for c in range(N_CORES):
        in_maps.append(
            {
                "xt": np.ascontiguousarray(
                    x2[:, c * T_PER_CORE:(c + 1) * T_PER_CORE]
                ),
                "wt": wt,
                "scale2": sc,
                "bias2": bi,
            }
        )
    return in_maps


def _assemble(results):
    # each core returns out [O, T_PER_CORE]; tokens are contiguous per core
    out = np.concatenate(
        [np.ascontiguousarray(r["out"].T) for r in results], axis=0
    )  # [TOKENS, O]
    return out.reshape(B, S, OUT_F)


def _run(x, weight_ternary, weight_scale, bias, trace=False, **spmd_kwargs):
    from concourse.bass_utils import run_bass_kernel_spmd

    nc = _get_nc()
    in_maps = _prep_inputs(x, weight_ternary, weight_scale, bias)
    res = run_bass_kernel_spmd(
        nc, in_maps, core_ids=list(range(N_CORES)), trace=trace, **spmd_kwargs
    )
    return _assemble(res.results), res


def kernel(x, weight_ternary, weight_scale, bias):
    out, _ = _run(x, weight_ternary, weight_scale, bias, trace=False)
    return out
